# revision 8
# baseline (speedup 1.0000x reference)
"""4-layer GATv2 forward pass on 8 TRN2 NeuronCores (Bass/Tile).

Strategy (node/dst partitioning, no cross-core segment reductions):
  - Nodes are padded to 20480 and split into 8 contiguous slices of 2560
    (20 blocks of 128 dst nodes per core).  Each core owns the segment
    softmax + weighted scatter for its dst nodes, so all softmax
    reductions are core-local.
  - Edges (with self loops appended) are routed to the core/block that
    owns their dst.  Per (core, block) edge counts are padded to a
    shared multiple of 128 (G[b] groups of 128 edges) so one NEFF works
    for all 8 cores.
  - Per layer: each core computes xl/xr for its 2560 nodes (PE matmuls,
    interleaved block-by-block with the previous layer's edge phase so
    PE fills gaps), the xl table is AllGather'ed (fp16, in 4 chunks so
    the collective overlaps the edge/matmul pipeline); per-edge work is
    edge-major (partition = edge % 128): one dma_gather per block for
    xl[src] rows and one for xr[dst] rows, z = xl+xr (DVE), leaky-relu
    (scalar), per-head dot with `a` (DVE folds+reduce), exp, then the
    softmax denominator and alpha-weighted sum of xl[src] via one PE
    matmul per 128-edge group against a host-precomputed one-hot dst
    selector (selt, zero columns for pad edges).
  - Softmax uses exp(logit) directly (no running max): logits are O(10)
    here, bf16/fp32 exp has the range, and the math is identical to the
    reference's shifted softmax.

kernel(**inputs) takes the full problem inputs and returns the full
[20000, 16] fp32 output.
"""

import numpy as np
import ml_dtypes

import concourse.bass as bass
import concourse.bacc as bacc
import concourse.mybir as mybir
import concourse.tile as tile
from concourse.tile import add_dep_helper
from concourse.bass_utils import run_bass_kernel_spmd
from concourse.masks import make_identity

F16 = mybir.dt.float16
BF16 = mybir.dt.bfloat16
F32 = mybir.dt.float32
I16 = mybir.dt.int16
P = 128

# model dims (fixed by the problem)
N_REAL = 20000
E_RAW = 320000
IN_CH = 128
HID = 64
HEADS = 4
OUT_CH = 16
SLOPE = 0.2

DEN_EPS = 1e-12   # keeps reciprocal() in range for edgeless (pad) dst rows
import os
AG_CHUNKS = int(os.environ.get("K_AG_CHUNKS", "4"))  # AllGather split


class Cfg:
    def __init__(self, n_cores, npc, n_real, layers, out_real):
        assert npc % P == 0
        self.n_cores = n_cores
        self.npc = npc              # nodes per core (padded)
        self.nblk = npc // P        # dst blocks per core
        self.n_real = n_real
        self.npad = n_cores * npc
        self.layers = layers        # list of dicts: c_in, c_tbl, n_h, c_h
        self.out_real = out_real    # real output channels of last layer


def real_cfg():
    layers = [
        dict(c_in=IN_CH, c_tbl=HEADS * HID, n_h=HEADS, c_h=HID),
        dict(c_in=HEADS * HID, c_tbl=HEADS * HID, n_h=HEADS, c_h=HID),
        dict(c_in=HEADS * HID, c_tbl=HEADS * HID, n_h=HEADS, c_h=HID),
        dict(c_in=HEADS * HID, c_tbl=P, n_h=1, c_h=P),  # 16 real, padded to 128
    ]
    return Cfg(8, 2560, N_REAL, layers, OUT_CH)


# ---------------------------------------------------------------------------
# host-side graph preprocessing
# ---------------------------------------------------------------------------

def prep_graph(cfg, edge_index):
    """Route edges (plus self loops) to (core, block) by dst; build per-core
    gather-index / one-hot-selector arrays in the exact layouts the kernel
    consumes."""
    n = cfg.n_real
    src = np.concatenate([np.asarray(edge_index[0], np.int64),
                          np.arange(n, dtype=np.int64)])
    dst = np.concatenate([np.asarray(edge_index[1], np.int64),
                          np.arange(n, dtype=np.int64)])
    assert src.min() >= 0 and src.max() < n and dst.min() >= 0 and dst.max() < n

    gblk = dst // P                       # global block id (core-major)
    order = np.argsort(gblk, kind="stable")
    src, dst, gblk = src[order], dst[order], gblk[order]

    nblk_tot = cfg.n_cores * cfg.nblk
    counts = np.bincount(gblk, minlength=nblk_tot).reshape(cfg.n_cores, cfg.nblk)
    G = np.maximum(1, (counts.max(axis=0) + P - 1) // P).astype(np.int64)  # [nblk]
    W = int(G.sum())

    # split edges per (core, block)
    starts = np.zeros(nblk_tot + 1, np.int64)
    np.cumsum(counts.reshape(-1), out=starts[1:])

    per_core = []
    for c in range(cfg.n_cores):
        xl_idx = np.zeros((P, 8 * W), np.int16)
        xr_idx = np.zeros((P, 8 * W), np.int16)
        selt = np.zeros((P, W * P), ml_dtypes.bfloat16)
        off = 0
        for b in range(cfg.nblk):
            gb = c * cfg.nblk + b
            s, e = starts[gb], starts[gb + 1]
            nreal = int(e - s)
            gG = int(G[b])
            npad_e = gG * P
            fsrc = np.zeros(npad_e, np.int64)
            fxr = np.zeros(npad_e, np.int64)
            # xl table is chunk-major ([chunk, core, row] with
            # npc/AG_CHUNKS rows per (chunk, core)) so each AllGather
            # chunk writes a contiguous slab; remap src ids to match.
            cr = cfg.npc // AG_CHUNKS
            s_core, s_loc = src[s:e] // cfg.npc, src[s:e] % cfg.npc
            fsrc[:nreal] = ((s_loc // cr) * (cfg.n_cores * cr)
                            + s_core * cr + s_loc % cr)
            fxr[:nreal] = dst[s:e] - c * cfg.npc
            # one-hot dst selector: selt[e%128, (off + e//128)*128 + dst%128]
            ee = np.arange(nreal)
            sl = np.zeros((P, gG * P), np.float32)
            sl[ee % P, (ee // P) * P + (dst[s:e] % P)] = 1.0
            selt[:, off * P:(off + gG) * P] = sl.astype(ml_dtypes.bfloat16)
            # wrapped idx layout: wrapped[p, s] = flat[s*16 + p], replicated
            # into all 8 16-partition groups (one per GPSIMD Q7 core)
            xl_idx[:, 8 * off:8 * (off + gG)] = np.tile(
                fsrc.astype(np.int16).reshape(-1, 16).T, (8, 1))
            xr_idx[:, 8 * off:8 * (off + gG)] = np.tile(
                fxr.astype(np.int16).reshape(-1, 16).T, (8, 1))
            off += gG
        per_core.append(dict(xl_idx=xl_idx, xr_idx=xr_idx, selt=selt))
    return [int(g) for g in G], per_core


# ---------------------------------------------------------------------------
# bass program
# ---------------------------------------------------------------------------

def build_nc(cfg, G):
    nl = len(cfg.layers)
    W = sum(G)
    Gmax = max(G)
    c_tbl_max = max(L["c_tbl"] for L in cfg.layers)
    kc_max = max(L["c_in"] for L in cfg.layers) // P
    ec_max = max(L["c_tbl"] + L["n_h"] for L in cfg.layers)

    nc = bacc.Bacc("TRN2", target_bir_lowering=False, debug=False,
                   num_devices=cfg.n_cores, num_swdge_queues=4)

    h0 = nc.dram_tensor("h0", [cfg.npc, cfg.layers[0]["c_in"]], F16,
                        kind="ExternalInput")
    xl_idx_d = nc.dram_tensor("xl_idx", [P, 8 * W], I16, kind="ExternalInput")
    xr_idx_d = nc.dram_tensor("xr_idx", [P, 8 * W], I16, kind="ExternalInput")
    selt_d = nc.dram_tensor("selt", [P, W * P], BF16, kind="ExternalInput")
    w_d, a_d = [], []
    for l, L in enumerate(cfg.layers):
        # [wl | wr] concatenated along the out dim, per 128-row kc chunk
        w_d.append(nc.dram_tensor(f"w{l}", [L["c_in"], 2 * L["c_tbl"]], F16,
                                  kind="ExternalInput"))
        a_d.append(nc.dram_tensor(f"a{l}", [P, L["c_tbl"]], F16,
                                  kind="ExternalInput"))
    out_d = nc.dram_tensor("out", [cfg.npc, cfg.out_real], F32,
                           kind="ExternalOutput")

    rg = [list(range(cfg.n_cores))]
    cpb = cfg.nblk // AG_CHUNKS          # blocks per AllGather chunk

    with tile.TileContext(nc) as tc:
        with (
            tc.tile_pool(name="const", bufs=1) as cpool,
            tc.tile_pool(name="wts", bufs=2) as wpool,
            tc.tile_pool(name="mm", bufs=2) as mpool,
            tc.tile_pool(name="gath", bufs=3) as gpool,
            tc.tile_pool(name="idx", bufs=4) as ipool,
            tc.tile_pool(name="edge", bufs=2) as epool,
            tc.tile_pool(name="small", bufs=2) as spool,
            tc.tile_pool(name="psum", bufs=2, space="PSUM") as ppool,
            tc.tile_pool(name="psumT", bufs=2, space="PSUM") as tpool,
            tc.tile_pool(name="dram", bufs=1, space="DRAM") as dpool,
        ):
            ident = cpool.tile([P, P], F16, tag="ident")
            make_identity(nc, ident[:])

            # ---- per-layer DRAM scratch -----------------------------------
            xl_loc, xr_loc, xl_tbl = [], [], []
            for l, L in enumerate(cfg.layers):
                xl_loc.append(dpool.tile([cfg.npc, L["c_tbl"]], F16,
                                         tag=f"xlloc{l}", name=f"xlloc{l}"))
                xr_loc.append(dpool.tile([cfg.npc, L["c_tbl"]], F16,
                                         tag=f"xrloc{l}", name=f"xrloc{l}"))
                # Shared DRAM allows only a single writer instruction, so
                # chunked AllGathers must use a Local table.
                xl_tbl.append(dpool.tile(
                    [cfg.npad, L["c_tbl"]], F16, tag=f"xltbl{l}",
                    name=f"xltbl{l}",
                    addr_space="Shared" if AG_CHUNKS == 1 else "Local"))

            last_g = [None]

            def gather_rows(tbl_ap, idx_dram, out_tile, off, gG, C_, nm, qn):
                """One dma_gather for a whole block (gG*128 rows)."""
                it = ipool.tile([P, 8 * Gmax], I16, tag="idxt",
                                name=f"idxt_{nm}")
                nc.sync.dma_start(out=it[:, :8 * gG],
                                  in_=idx_dram[:, 8 * off:8 * (off + gG)])
                gi = nc.gpsimd.dma_gather(
                    out_ap=out_tile[:, :gG * C_]
                        .rearrange("p (g c) -> p g c", c=C_),
                    in_ap=tbl_ap,
                    idxs_ap=it[:, :8 * gG],
                    num_idxs=gG * P, num_idxs_reg=gG * P,
                    elem_size=C_, queue_num=qn)
                if last_g[0] is not None:
                    add_dep_helper(gi.ins, last_g[0].ins, sync=False,
                                   reason="pin gather order")
                last_g[0] = gi

            def load_weights(l):
                L = cfg.layers[l]
                c_in, C = L["c_in"], L["c_tbl"]
                kc_n = c_in // P
                wlr = wpool.tile([P, kc_max * 2 * c_tbl_max], F16, tag="wlr")
                for kc in range(kc_n):
                    nc.sync.dma_start(
                        out=wlr[:, kc * 2 * C:(kc + 1) * 2 * C],
                        in_=w_d[l][kc * P:(kc + 1) * P, :])
                a_sb = wpool.tile([P, c_tbl_max], F16, tag="a_sb")
                nc.sync.dma_start(out=a_sb[:, :C], in_=a_d[l][:])
                return wlr, a_sb

            def mm_block(l, b, hb_tile):
                """xl/xr transform for this core's dst block b of layer l.
                hb_tile: SBUF [P, c_in] fp16 (or None -> load from h0)."""
                L = cfg.layers[l]
                c_in, C = L["c_in"], L["c_tbl"]
                kc_n = c_in // P
                wlr = w_sb[l]
                if hb_tile is None:
                    hb_tile = mpool.tile([P, c_in], F16, tag="h0t")
                    nc.sync.dma_start(out=hb_tile[:],
                                      in_=h0[b * P:(b + 1) * P, :])
                hT = mpool.tile([P, kc_max * P], F16, tag="hT")
                for kc in range(kc_n):
                    pt = tpool.tile([P, P], F16, tag="pt")
                    nc.tensor.transpose(pt[:], hb_tile[:, kc * P:(kc + 1) * P],
                                        ident[:])
                    nc.scalar.activation(hT[:, kc * P:(kc + 1) * P], pt[:],
                                         mybir.ActivationFunctionType.Copy)
                ps = ppool.tile([P, 2 * c_tbl_max], F32, tag="ps_mm")
                for kc in range(kc_n):
                    nc.tensor.matmul(ps[:, :2 * C],
                                     lhsT=hT[:, kc * P:(kc + 1) * P],
                                     rhs=wlr[:, kc * 2 * C:(kc + 1) * 2 * C],
                                     start=(kc == 0), stop=(kc == kc_n - 1))
                xlr_t = mpool.tile([P, 2 * c_tbl_max], F16, tag="xlr_t")
                nc.scalar.activation(xlr_t[:, :2 * C], ps[:, :2 * C],
                                     mybir.ActivationFunctionType.Copy)
                nc.sync.dma_start(out=xl_loc[l][b * P:(b + 1) * P, :],
                                  in_=xlr_t[:, :C])
                nc.sync.dma_start(out=xr_loc[l][b * P:(b + 1) * P, :],
                                  in_=xlr_t[:, C:2 * C])

            def ag_chunk(l, k):
                """AllGather chunk k (cpb blocks) of layer l's xl table.
                The table is chunk-major ([chunk, core, row]) so the output
                slab is contiguous (a BIR requirement for collectives)."""
                r0, r1 = k * cpb * P, (k + 1) * cpb * P
                t0 = k * cfg.n_cores * cpb * P
                t1 = (k + 1) * cfg.n_cores * cpb * P
                nc.gpsimd.collective_compute(
                    "AllGather", mybir.AluOpType.bypass, replica_groups=rg,
                    ins=[xl_loc[l][r0:r1, :].opt()],
                    outs=[xl_tbl[l][t0:t1, :].opt()])

            def edge_block(l, b):
                """Per-edge phase for dst block b of layer l. Returns hb
                (SBUF [P, C] fp16 relu'd output) or None for the last layer
                (which DMAs to out_d)."""
                L = cfg.layers[l]
                C, n_h, c_h = L["c_tbl"], L["n_h"], L["c_h"]
                EC = C + n_h
                gG = G[b]
                off = sum(G[:b])
                ch2, ch4 = c_h // 2, c_h // 4

                xl_g = gpool.tile([P, Gmax * c_tbl_max], F16, tag="xl_g")
                gather_rows(xl_tbl[l][:, :], xl_idx_d, xl_g, off, gG, C,
                            f"xl{l}_{b}", (2 * b) % 4)
                xr_g = gpool.tile([P, Gmax * c_tbl_max], F16, tag="xr_g")
                gather_rows(xr_loc[l][:, :], xr_idx_d, xr_g, off, gG, C,
                            f"xr{l}_{b}", (2 * b + 1) % 4)
                selt = gpool.tile([P, Gmax * P], BF16, tag="selt")
                nc.sync.dma_start(out=selt[:, :gG * P],
                                  in_=selt_d[:, off * P:(off + gG) * P])

                # z = xl[src] + xr[dst]; leaky relu on scalar engine
                z = epool.tile([P, Gmax * c_tbl_max], F16, tag="z")
                nc.vector.tensor_tensor(out=z[:, :gG * C],
                                        in0=xl_g[:, :gG * C],
                                        in1=xr_g[:, :gG * C],
                                        op=mybir.AluOpType.add)
                lrz = epool.tile([P, Gmax * c_tbl_max], F16, tag="lrz")
                nc.scalar.activation(lrz[:, :gG * C], z[:, :gG * C],
                                     mybir.ActivationFunctionType.Prelu,
                                     alpha=SLOPE)
                # a * LR(z), then fold+reduce to per-(edge, head) logits
                alr = epool.tile([P, Gmax * c_tbl_max], F16, tag="alr")
                nc.vector.tensor_tensor(
                    out=alr[:, :gG * C].rearrange("p (g c) -> p g c", c=C),
                    in0=lrz[:, :gG * C].rearrange("p (g c) -> p g c", c=C),
                    in1=w_a[l][:, :C].rearrange("p (g c) -> p g c", g=1)
                        .to_broadcast([P, gG, C]),
                    op=mybir.AluOpType.mult)
                fold1 = spool.tile([P, Gmax * c_tbl_max // 2], F16, tag="fold1")
                a4 = alr[:, :gG * C].rearrange("p (g h c) -> p g h c",
                                               h=n_h, c=c_h)
                f13 = fold1[:, :gG * C // 2].rearrange(
                    "p (g h c) -> p g h c", h=n_h, c=ch2)
                nc.vector.tensor_tensor(out=f13, in0=a4[:, :, :, :ch2],
                                        in1=a4[:, :, :, ch2:],
                                        op=mybir.AluOpType.add)
                fold2 = spool.tile([P, Gmax * c_tbl_max // 4], F16, tag="fold2")
                f23 = fold2[:, :gG * C // 4].rearrange(
                    "p (g h c) -> p g h c", h=n_h, c=ch4)
                nc.vector.tensor_tensor(out=f23, in0=f13[:, :, :, :ch4],
                                        in1=f13[:, :, :, ch4:],
                                        op=mybir.AluOpType.add)
                logits = spool.tile([P, Gmax * HEADS], F32, tag="logits")
                nc.vector.tensor_reduce(
                    out=logits[:, :gG * n_h].rearrange("p (g h) -> p g h",
                                                       h=n_h),
                    in_=f23, axis=mybir.AxisListType.X, op=mybir.AluOpType.add)
                ex = spool.tile([P, Gmax * HEADS], BF16, tag="ex")
                nc.scalar.activation(ex[:, :gG * n_h], logits[:, :gG * n_h],
                                     mybir.ActivationFunctionType.Exp)
                # edata = [ex * xl[src] | ex]  (ex broadcast over c_h on DVE)
                edata = epool.tile([P, Gmax * ec_max], BF16, tag="edata")
                ed4 = edata[:, :gG * EC].rearrange("p (g c) -> p g c", c=EC)
                nc.vector.tensor_tensor(
                    out=ed4[:, :, :C].rearrange("p g (h c) -> p g h c",
                                                c=c_h),
                    in0=xl_g[:, :gG * C].rearrange("p (g h c) -> p g h c",
                                                   h=n_h, c=c_h),
                    in1=ex[:, :gG * n_h].rearrange("p (g h c) -> p g h c",
                                                   h=n_h, c=1)
                        .to_broadcast([P, gG, n_h, c_h]),
                    op=mybir.AluOpType.mult)
                nc.vector.tensor_copy(
                    out=ed4[:, :, C:],
                    in_=ex[:, :gG * n_h].rearrange("p (g h) -> p g h", h=n_h))
                # segment sums via PE: psum[d, :] += selt_g^T @ edata_g
                ps_nd = ppool.tile([P, ec_max], F32, tag="ps_nd")
                for g in range(gG):
                    nc.tensor.matmul(
                        ps_nd[:, :EC], lhsT=selt[:, g * P:(g + 1) * P],
                        rhs=edata[:, g * EC:(g + 1) * EC],
                        start=(g == 0), stop=(g == gG - 1))
                den = spool.tile([P, HEADS], F32, tag="den")
                nc.vector.tensor_scalar(
                    out=den[:, :n_h], in0=ps_nd[:, C:EC], scalar1=DEN_EPS,
                    scalar2=None, op0=mybir.AluOpType.add)
                rden = spool.tile([P, HEADS], F32, tag="rden")
                nc.vector.reciprocal(rden[:, :n_h], den[:, :n_h])
                ob = spool.tile([P, c_tbl_max], F32, tag="ob")
                nc.vector.tensor_tensor(
                    out=ob[:, :C].rearrange("p (h c) -> p h c", h=n_h),
                    in0=ps_nd[:, :C].rearrange("p (h c) -> p h c", h=n_h),
                    in1=rden[:, :n_h].rearrange("p (h c) -> p h c", c=1)
                        .to_broadcast([P, n_h, c_h]),
                    op=mybir.AluOpType.mult)
                if l + 1 < nl:
                    hb = spool.tile([P, c_tbl_max], F16, tag="hb")
                    nc.scalar.activation(hb[:, :C], ob[:, :C],
                                         mybir.ActivationFunctionType.Relu)
                    return hb
                nc.sync.dma_start(out=out_d[b * P:(b + 1) * P, :],
                                  in_=ob[:, :cfg.out_real])
                return None

            # ---- program ---------------------------------------------------
            w_sb, w_a = [None] * nl, [None] * nl
            w_sb[0], w_a[0] = load_weights(0)
            # prologue: layer-0 transform + chunked AllGather
            for b in range(cfg.nblk):
                mm_block(0, b, None)
                if (b + 1) % cpb == 0:
                    ag_chunk(0, b // cpb)

            for l in range(nl):
                if l + 1 < nl:
                    w_sb[l + 1], w_a[l + 1] = load_weights(l + 1)
                for b in range(cfg.nblk):
                    # delayed AllGather triggers so the GpSimd queue doesn't
                    # stall gather dispatches while waiting on mm writes
                    if l + 1 < nl and b >= cpb + 2 and (b - 2) % cpb == 0:
                        ag_chunk(l + 1, (b - 2) // cpb - 1)
                    hb = edge_block(l, b)
                    if l + 1 < nl:
                        mm_block(l + 1, b, hb)
                if l + 1 < nl:
                    ag_chunk(l + 1, AG_CHUNKS - 1)
    nc.compile()
    return nc


# ---------------------------------------------------------------------------
# host orchestration
# ---------------------------------------------------------------------------

def _wlr_pad(wl, wr, c_tbl):
    """wl/wr: [h*oc, ic] fp32 -> [ic, 2*c_tbl] fp16 ([wl | wr], zero-padded
    out channels)."""
    wl = np.asarray(wl, np.float32)
    wr = np.asarray(wr, np.float32)
    hoc, ic = wl.shape
    out = np.zeros((ic, 2 * c_tbl), np.float16)
    out[:, :hoc] = wl.T.astype(np.float16)
    out[:, c_tbl:c_tbl + hoc] = wr.T.astype(np.float16)
    return out


def _a_rep(a, c_tbl):
    """a: [h, oc] fp32 -> [128, c_tbl] fp16 replicated across partitions."""
    a = np.asarray(a, np.float32).reshape(-1)
    row = np.zeros(c_tbl, np.float16)
    row[:a.shape[0]] = a.astype(np.float16)
    return np.tile(row[None, :], (P, 1))


def make_in_maps(cfg, G, per_core, x, weights):
    xpad = np.zeros((cfg.npad, cfg.layers[0]["c_in"]), np.float16)
    xpad[:cfg.n_real] = np.asarray(x, np.float32).astype(np.float16)
    shared = {}
    for l, L in enumerate(cfg.layers):
        wl, wr, a = weights[l]
        shared[f"w{l}"] = _wlr_pad(wl, wr, L["c_tbl"])
        shared[f"a{l}"] = _a_rep(a, L["c_tbl"])
    in_maps = []
    for c in range(cfg.n_cores):
        m = dict(shared)
        m["h0"] = xpad[c * cfg.npc:(c + 1) * cfg.npc]
        m["xl_idx"] = per_core[c]["xl_idx"]
        m["xr_idx"] = per_core[c]["xr_idx"]
        m["selt"] = per_core[c]["selt"]
        in_maps.append(m)
    return in_maps


_CACHE = {}


def _get_built(cfg, edge_index):
    key = hash(np.asarray(edge_index).tobytes())
    if key not in _CACHE:
        G, per_core = prep_graph(cfg, edge_index)
        nc = build_nc(cfg, G)
        _CACHE[key] = (G, per_core, nc)
    return _CACHE[key]


def kernel(x, edge_index,
           w1l, b1l, w1r, b1r, a1, bo1,
           w2l, b2l, w2r, b2r, a2, bo2,
           w3l, b3l, w3r, b3r, a3, bo3,
           w4l, b4l, w4r, b4r, a4, bo4,
           _trace=False):
    cfg = real_cfg()
    for b in (b1l, b1r, b2l, b2r, b3l, b3r, b4l, b4r, bo1, bo2, bo3):
        assert np.max(np.abs(np.asarray(b, np.float32))) == 0.0, \
            "non-zero internal biases not supported"
    G, per_core, nc = _get_built(cfg, edge_index)
    weights = [(w1l, w1r, a1), (w2l, w2r, a2), (w3l, w3r, a3), (w4l, w4r, a4)]
    in_maps = make_in_maps(cfg, G, per_core, x, weights)
    res = run_bass_kernel_spmd(nc, in_maps, core_ids=list(range(cfg.n_cores)),
                               trace=_trace)
    outs = [np.asarray(res.results[c]["out"]) for c in range(cfg.n_cores)]
    full = np.concatenate(outs, axis=0)[:cfg.n_real].astype(np.float32)
    full = full + np.asarray(bo4, np.float32)[None, :]
    if _trace:
        kernel.last_exec_time_ns = res.exec_time_ns
        kernel.last_res = res
    return full


kernel.last_exec_time_ns = None
kernel.last_res = None
